# revision 24
# baseline (speedup 1.0000x reference)
# Binary linear: y[b,s,o] = sum_i x[b,s,i] * sign(W)[o,i]
#
# Strategy (8 NeuronCores, data-parallel over tokens):
#   - Host: flatten x to [32768, 768], shard 8 x [4096, 768], pre-transpose
#     each shard to xT [768, 4096]; split x into fp8 e4m3 parts:
#     hi = e4m3(x) over all 6 contraction blocks, lo = e4m3(x - hi) over
#     the first 4 blocks only. Weights are exactly +-1, so only x carries
#     quantization error; correcting 4/6 of the contraction gives
#     rel err 1.55e-2 (< 2e-2 gate) while cutting PE work to 5/6 of the
#     bf16 stream.
#   - Device (per core): fp8 DoubleRow matmuls — each PE instruction
#     consumes TWO 128-deep contraction tiles (lhsT [p,2,128] stationary,
#     rhs [p,2,N] moving) in N cycles, so the 10 k-tiles (6 hi + 4 lo)
#     per 128-token block cost 5 instructions (vs 6 for bf16):
#       psum[t,o] += sum_j xT_pair[i,j,t].T @ wbinT_pair[i,j,o]
#     PE-stream floor: 32 blocks * 5 * 768 cols = 51.2 us.
#   - Data path: everything is pre-packed on the host so each DMA is 128
#     fat per-partition-contiguous descriptors. x chunks (hi+lo fused,
#     [128, 10, cw]) ride the sync HWDGE ring; the weight tiles ride the
#     scalar ring, split into the 512-col half (wA, needed by matmul #1)
#     and the 256-col half (wB, needed ~2us later) so the PE can start as
#     soon as ~300KB have landed. Graduated chunk widths keep the feed
#     just-in-time while the HAM clock ramps.
#   - y copied out of PSUM as bf16 (halves store traffic), upcast on host.
#   - Host: concat shards -> [4, 8192, 768] f32.

import numpy as np

N_CORES = 8
B, S, D_IN, D_OUT = 4, 8192, 768, 768
T_TOTAL = B * S            # 32768 tokens
T_CORE = T_TOTAL // N_CORES  # 4096 tokens per core
P = 128
IB = D_IN // P             # 6 i-blocks (contraction)
MP = IB // 2               # 3 i-block pairs (DoubleRow)
LB = 4                     # lo-corrected i-blocks (first 4 of 6)
LP = LB // 2               # 2 lo pairs
NB = IB + LB               # 10 k-blocks per token chunk (6 hi + 4 lo)
TB = T_CORE // P           # 32 token-blocks per core
# graduated chunk widths: small first chunks so the PE can start early,
# large later chunks for DMA efficiency.
CHUNKS = [128, 256, 384, 768, 1024, 1536]
assert sum(CHUNKS) == T_CORE
O_SPLIT = 512              # split for PSUM banks / copies
N_WARMUP = 34              # N=128 warmup matmuls to ramp the HAM clock

_cache = {}


def _build():
    import concourse.bacc as bacc
    import concourse.mybir as mybir
    import concourse.tile as tile

    f32 = mybir.dt.float32
    bf16 = mybir.dt.bfloat16
    fp8 = mybir.dt.float8e4
    DR = mybir.MatmulPerfMode.DoubleRow

    nc = bacc.Bacc(
        "TRN2",
        target_bir_lowering=False,
        debug=False,
        num_devices=N_CORES,
    )

    xp = nc.dram_tensor("xp", [P, NB * T_CORE], fp8, kind="ExternalInput")
    wa = nc.dram_tensor("wa", [P, IB * O_SPLIT], fp8, kind="ExternalInput")
    wb = nc.dram_tensor(
        "wb", [P, IB * (D_OUT - O_SPLIT)], fp8, kind="ExternalInput"
    )
    y = nc.dram_tensor("y", [T_CORE, D_OUT], bf16, kind="ExternalOutput")

    with tile.TileContext(nc) as tc:
        with (
            tc.tile_pool(name="wbin", bufs=1) as wbin_pool,
            tc.tile_pool(name="xbuf", bufs=1) as x_pool,
            tc.tile_pool(name="ybuf", bufs=8) as y_pool,
            tc.tile_pool(name="psum", bufs=3, space="PSUM") as psum_pool,
        ):
            chunk_start = []
            s = 0
            for w_ in CHUNKS:
                chunk_start.append(s)
                s += w_

            # --- PE warmup: dummy matmuls on a small zeroed tile during the
            # framework preamble / first DMAs, so the HAM clock gate is near
            # full rate when the real matmuls start. ---
            wu = x_pool.tile([P, P], bf16, tag="warmup", name="wu")
            nc.gpsimd.memset(wu[:], 0.0)
            wups = psum_pool.tile([P, P], f32, tag="wups", name="wups", bufs=1)
            for k in range(N_WARMUP):
                nc.tensor.matmul(
                    wups[:], wu[:], wu[:],
                    start=True, stop=True, skip_group_check=True,
                )
            wu_out = x_pool.tile([P, P], f32, tag="warmup_out", name="wu_out")
            nc.vector.tensor_copy(wu_out[:], wups[:])

            xch = [None] * len(CHUNKS)

            def x_load(c, eng):
                cw = CHUNKS[c]
                c0 = chunk_start[c]
                xt = x_pool.tile([P, NB, cw], fp8, tag=f"xch{c}", name=f"xch{c}")
                eng.dma_start(
                    xt[:],
                    xp[:, NB * c0 : NB * (c0 + cw)].rearrange(
                        "p (b t) -> p b t", b=NB
                    ),
                )
                xch[c] = xt

            # weight halves: one tile each, the wA half split into three
            # pair-loads so matmul #k waits only on pair k's 128KB.
            wat = wbin_pool.tile([P, IB, O_SPLIT], fp8, tag="wa", name="wat")
            wbt = wbin_pool.tile(
                [P, IB, D_OUT - O_SPLIT], fp8, tag="wb", name="wbt"
            )

            # weights lead the sync HWDGE FIFO (the first matmuls gate on
            # them); x chunks ride the scalar ring in parallel. Only the
            # small chunks are issued up front: the 16 SDMA engines
            # round-robin across ALL in-flight transfers, so a big chunk
            # in flight would starve the weight loads. The big chunks are
            # issued mid-loop, throttled behind the y-store descriptor
            # gens (which wait on the stream's copy semaphores).
            nc.sync.dma_start(
                wat[:], wa[:].rearrange("p (b o) -> p b o", b=IB)
            )
            nc.sync.dma_start(
                wbt[:], wb[:].rearrange("p (b o) -> p b o", b=IB)
            )
            x_load(0, nc.scalar)
            x_load(1, nc.scalar)
            x_load(2, nc.scalar)

            def chunk_of(tok):
                for c in range(len(CHUNKS) - 1, -1, -1):
                    if tok >= chunk_start[c]:
                        return c, tok - chunk_start[c]
                raise AssertionError

            # --- main loop: one 128-token block at a time ---
            for j in range(TB):
                if j == 2:
                    x_load(3, nc.sync)
                elif j == 6:
                    x_load(4, nc.sync)
                elif j == 14:
                    x_load(5, nc.sync)
                c, off = chunk_of(j * P)
                cw = CHUNKS[c]
                xc = xch[c]
                ps = psum_pool.tile([P, D_OUT], f32, tag="ps", name=f"ps{j}")
                yt = y_pool.tile([P, D_OUT], bf16, tag="y", name=f"y{j}")
                tail = j >= TB - 2

                # schedule: hi pair 2 goes LAST so its weights (3rd in the
                # scalar ring's FIFO) have the most slack at startup.
                # (xc rows 0-5 = hi blocks, 6-9 = lo blocks.)
                SCHED = ((0, 0), (2, 1), (6, 0), (8, 1), (4, 2))

                # all matmuls of the 512-column half first, then the
                # 256-half: the 512-half PSUM group closes earlier so its
                # (bigger) copy-out overlaps the 256-half matmuls.
                def half(lo, hi_, wt):
                    for k, (row, m) in enumerate(SCHED):
                        nc.tensor.matmul(
                            ps[:, lo:hi_],
                            xc[:, row : row + 2, off : off + P],
                            wt[:, 2 * m : 2 * m + 2, :],
                            start=(k == 0),
                            stop=(k == len(SCHED) - 1),
                            perf_mode=DR,
                        )

                half(0, O_SPLIT, wat)
                if tail:
                    # tail: the 512-half copy+store launches while the PE
                    # still runs the 256-half matmuls; the (smaller)
                    # 256-half chain rides the ACT engine + its own ring,
                    # so only ~1us of copy/store latency trails the last
                    # matmul.
                    nc.vector.tensor_copy(yt[:, :O_SPLIT], ps[:, :O_SPLIT])
                    nc.sync.dma_start(
                        y[j * P : (j + 1) * P, :O_SPLIT], yt[:, :O_SPLIT]
                    )
                half(O_SPLIT, D_OUT, wbt)
                if tail:
                    # drain the final 256 columns as two 128-col pieces on
                    # parallel engines + rings, so the post-stream chain is
                    # one small copy + one descriptor-gen deep.
                    MID = O_SPLIT + 128
                    nc.scalar.copy(yt[:, O_SPLIT:MID], ps[:, O_SPLIT:MID])
                    nc.scalar.dma_start(
                        y[j * P : (j + 1) * P, O_SPLIT:MID], yt[:, O_SPLIT:MID]
                    )
                    nc.vector.tensor_copy(yt[:, MID:], ps[:, MID:])
                    nc.sync.dma_start(
                        y[j * P : (j + 1) * P, MID:], yt[:, MID:]
                    )
                else:
                    nc.vector.tensor_copy(yt[:, :O_SPLIT], ps[:, :O_SPLIT])
                    nc.scalar.copy(yt[:, O_SPLIT:], ps[:, O_SPLIT:])
                    eng = nc.sync if j % 2 == 0 else nc.scalar
                    eng.dma_start(y[j * P : (j + 1) * P, :], yt[:])

    nc.compile()
    return nc


def _get_nc():
    if "nc" not in _cache:
        _cache["nc"] = _build()
    return _cache["nc"]


def _pack_chunks(a):
    """[cores, blocks, 128, T] -> [cores, 128, blocks*T] with each token
    chunk's data contiguous per partition row (chunk-major, then
    block-major, token-minor) so chunk loads are 128 fat descriptors."""
    nco, nb, p, t = a.shape
    pieces = []
    s = 0
    for cw in CHUNKS:
        pieces.append(a[:, :, :, s : s + cw].transpose(0, 2, 1, 3)
                      .reshape(nco, p, nb * cw))
        s += cw
    return np.ascontiguousarray(np.concatenate(pieces, axis=2))


def _prep_inputs(x, weight):
    import ml_dtypes

    f8 = ml_dtypes.float8_e4m3
    x = np.asarray(x, dtype=np.float32)
    w = np.asarray(weight, dtype=np.float32)
    x2 = x.reshape(N_CORES, T_CORE, D_IN)
    # transpose so the contraction dim is on partitions, then hi/lo fp8 split
    xT = np.ascontiguousarray(x2.transpose(0, 2, 1))  # [8, 768, 4096] f32
    xT_hi = xT.astype(f8)
    xT_lo = (
        (xT[:, : LB * P, :] - xT_hi[:, : LB * P, :].astype(np.float32))
    ).astype(f8)
    comb = np.concatenate(
        [
            xT_hi.reshape(N_CORES, IB, P, T_CORE),
            xT_lo.reshape(N_CORES, LB, P, T_CORE),
        ],
        axis=1,
    )  # [cores, 10, 128, T]
    xp_packed = _pack_chunks(comb)
    # replicate the small binarized weight: +-1 (and 0) are exact in fp8.
    # pack partition-major, split into the 512- and 256-col halves.
    wT = np.sign(w).T.astype(f8)  # [i, o]
    wa = np.ascontiguousarray(
        wT[:, :O_SPLIT].reshape(IB, P, O_SPLIT).transpose(1, 0, 2)
        .reshape(P, IB * O_SPLIT)
    )
    wb = np.ascontiguousarray(
        wT[:, O_SPLIT:].reshape(IB, P, D_OUT - O_SPLIT).transpose(1, 0, 2)
        .reshape(P, IB * (D_OUT - O_SPLIT))
    )
    return [
        {"xp": xp_packed[c], "wa": wa, "wb": wb} for c in range(N_CORES)
    ]


def _install_axon_ntff_hook():
    """The agent image's `antenv` lacks `axon_hooks`; register an equivalent
    module backed by direct ctypes calls into libaxon_pjrt.so so that
    run_bass_kernel_spmd(trace=True) can capture NTFF profiles under axon."""
    import sys

    if "antenv.axon_hooks" in sys.modules:
        return
    import contextlib
    import ctypes
    import types

    so_path = "/opt/axon/libaxon_pjrt.so"
    try:
        lib = ctypes.CDLL(so_path)
    except OSError:
        return
    if not hasattr(lib, "axon_start_nrt_profile"):
        return
    lib.axon_start_nrt_profile.argtypes = [
        ctypes.POINTER(ctypes.c_int64),
        ctypes.c_size_t,
    ]
    lib.axon_start_nrt_profile.restype = ctypes.c_int64
    lib.axon_stop_nrt_profile.argtypes = [ctypes.c_char_p]
    lib.axon_stop_nrt_profile.restype = ctypes.c_int64

    @contextlib.contextmanager
    def _hook(output_dir, device_ids):
        import jax

        jax.devices()
        if device_ids:
            ids = (ctypes.c_int64 * len(device_ids))(*device_ids)
            rc = lib.axon_start_nrt_profile(ids, len(device_ids))
        else:
            rc = lib.axon_start_nrt_profile(None, 0)
        if rc != 0:
            raise RuntimeError(f"axon_start_nrt_profile rc={rc}")
        try:
            yield
        finally:
            n = lib.axon_stop_nrt_profile(str(output_dir).encode())
            print(f"ntff profile: {n} file(s) written to {output_dir}")

    mod = types.ModuleType("antenv.axon_hooks")
    mod.get_axon_ntff_profile_hook = lambda: _hook
    mod.set_axon_ntff_profile_hook = lambda h: None
    sys.modules["antenv.axon_hooks"] = mod


def _run(x, weight, trace=False):
    from concourse.bass_utils import run_bass_kernel_spmd

    if trace:
        _install_axon_ntff_hook()
    nc = _get_nc()
    in_maps = _prep_inputs(x, weight)
    res = run_bass_kernel_spmd(
        nc, in_maps, core_ids=list(range(N_CORES)), trace=trace
    )
    y_full = np.concatenate(
        [r["y"].astype(np.float32) for r in res.results], axis=0
    )
    return y_full.reshape(B, S, D_OUT), res


def kernel(x, weight):
    out, _ = _run(x, weight, trace=False)
    return out


# revision 25
# speedup vs baseline: 1.0286x; 1.0286x over previous
# Binary linear: y[b,s,o] = sum_i x[b,s,i] * sign(W)[o,i]
#
# Strategy (8 NeuronCores, data-parallel over tokens):
#   - Host: flatten x to [32768, 768], shard 8 x [4096, 768], pre-transpose
#     each shard to xT [768, 4096]; split x into fp8 e4m3 parts:
#     hi = e4m3(x) over all 6 contraction blocks, lo = e4m3(x - hi) over
#     the first 4 blocks only. Weights are exactly +-1, so only x carries
#     quantization error; correcting 4/6 of the contraction gives
#     rel err 1.55e-2 (< 2e-2 gate) while cutting PE work to 5/6 of the
#     bf16 stream.
#   - Device (per core): fp8 DoubleRow matmuls — each PE instruction
#     consumes TWO 128-deep contraction tiles (lhsT [p,2,128] stationary,
#     rhs [p,2,N] moving) in N cycles, so the 10 k-tiles (6 hi + 4 lo)
#     per 128-token block cost 5 instructions (vs 6 for bf16):
#       psum[t,o] += sum_j xT_pair[i,j,t].T @ wbinT_pair[i,j,o]
#     PE-stream floor: 32 blocks * 5 * 768 cols = 51.2 us.
#   - Data path: everything is pre-packed on the host so each DMA is 128
#     fat per-partition-contiguous descriptors. x chunks (hi+lo fused,
#     [128, 10, cw]) ride the sync HWDGE ring; the weight tiles ride the
#     scalar ring, split into the 512-col half (wA, needed by matmul #1)
#     and the 256-col half (wB, needed ~2us later) so the PE can start as
#     soon as ~300KB have landed. Graduated chunk widths keep the feed
#     just-in-time while the HAM clock ramps.
#   - y copied out of PSUM as bf16 (halves store traffic), upcast on host.
#   - Host: concat shards -> [4, 8192, 768] f32.

import numpy as np

N_CORES = 8
B, S, D_IN, D_OUT = 4, 8192, 768, 768
T_TOTAL = B * S            # 32768 tokens
T_CORE = T_TOTAL // N_CORES  # 4096 tokens per core
P = 128
IB = D_IN // P             # 6 i-blocks (contraction)
MP = IB // 2               # 3 i-block pairs (DoubleRow)
LB = 4                     # lo-corrected i-blocks (first 4 of 6)
LP = LB // 2               # 2 lo pairs
NB = IB + LB               # 10 k-blocks per token chunk (6 hi + 4 lo)
TB = T_CORE // P           # 32 token-blocks per core
# graduated chunk widths: small first chunks so the PE can start early,
# large later chunks for DMA efficiency.
CHUNKS = [128, 256, 384, 768, 1024, 1536]
assert sum(CHUNKS) == T_CORE
O_SPLIT = 512              # split for PSUM banks / copies
N_WARMUP = 30              # N=128 warmup matmuls to ramp the HAM clock

_cache = {}


def _build():
    import concourse.bacc as bacc
    import concourse.mybir as mybir
    import concourse.tile as tile

    f32 = mybir.dt.float32
    bf16 = mybir.dt.bfloat16
    fp8 = mybir.dt.float8e4
    DR = mybir.MatmulPerfMode.DoubleRow

    nc = bacc.Bacc(
        "TRN2",
        target_bir_lowering=False,
        debug=False,
        num_devices=N_CORES,
    )

    xp = nc.dram_tensor("xp", [P, NB * T_CORE], fp8, kind="ExternalInput")
    wa = nc.dram_tensor("wa", [P, IB * O_SPLIT], fp8, kind="ExternalInput")
    wb = nc.dram_tensor(
        "wb", [P, IB * (D_OUT - O_SPLIT)], fp8, kind="ExternalInput"
    )
    y = nc.dram_tensor("y", [T_CORE, D_OUT], bf16, kind="ExternalOutput")

    with tile.TileContext(nc) as tc:
        with (
            tc.tile_pool(name="wbin", bufs=1) as wbin_pool,
            tc.tile_pool(name="xbuf", bufs=1) as x_pool,
            tc.tile_pool(name="ybuf", bufs=8) as y_pool,
            tc.tile_pool(name="psum", bufs=3, space="PSUM") as psum_pool,
        ):
            chunk_start = []
            s = 0
            for w_ in CHUNKS:
                chunk_start.append(s)
                s += w_

            # --- PE warmup: dummy matmuls on a small zeroed tile during the
            # framework preamble / first DMAs, so the HAM clock gate is near
            # full rate when the real matmuls start. ---
            wu = x_pool.tile([P, P], bf16, tag="warmup", name="wu")
            nc.gpsimd.memset(wu[:], 0.0)
            wups = psum_pool.tile([P, P], f32, tag="wups", name="wups", bufs=1)
            for k in range(N_WARMUP):
                nc.tensor.matmul(
                    wups[:], wu[:], wu[:],
                    start=True, stop=True, skip_group_check=True,
                )
            wu_out = x_pool.tile([P, P], f32, tag="warmup_out", name="wu_out")
            nc.vector.tensor_copy(wu_out[:], wups[:])

            xch = [None] * len(CHUNKS)

            def x_load(c, eng):
                cw = CHUNKS[c]
                c0 = chunk_start[c]
                xt = x_pool.tile([P, NB, cw], fp8, tag=f"xch{c}", name=f"xch{c}")
                eng.dma_start(
                    xt[:],
                    xp[:, NB * c0 : NB * (c0 + cw)].rearrange(
                        "p (b t) -> p b t", b=NB
                    ),
                )
                xch[c] = xt

            # weight halves: one tile each, the wA half split into three
            # pair-loads so matmul #k waits only on pair k's 128KB.
            wat = wbin_pool.tile([P, IB, O_SPLIT], fp8, tag="wa", name="wat")
            wbt = wbin_pool.tile(
                [P, IB, D_OUT - O_SPLIT], fp8, tag="wb", name="wbt"
            )

            # weights lead the sync HWDGE FIFO (the first matmuls gate on
            # them); x chunks ride the scalar ring in parallel. Only the
            # small chunks are issued up front: the 16 SDMA engines
            # round-robin across ALL in-flight transfers, so a big chunk
            # in flight would starve the weight loads. The big chunks are
            # issued mid-loop, throttled behind the y-store descriptor
            # gens (which wait on the stream's copy semaphores).
            nc.sync.dma_start(
                wat[:], wa[:].rearrange("p (b o) -> p b o", b=IB)
            )
            nc.sync.dma_start(
                wbt[:], wb[:].rearrange("p (b o) -> p b o", b=IB)
            )
            x_load(0, nc.scalar)
            x_load(1, nc.scalar)
            x_load(2, nc.scalar)

            def chunk_of(tok):
                for c in range(len(CHUNKS) - 1, -1, -1):
                    if tok >= chunk_start[c]:
                        return c, tok - chunk_start[c]
                raise AssertionError

            # --- main loop: one 128-token block at a time ---
            for j in range(TB):
                if j == 2:
                    x_load(3, nc.sync)
                elif j == 6:
                    x_load(4, nc.sync)
                elif j == 14:
                    x_load(5, nc.sync)
                c, off = chunk_of(j * P)
                cw = CHUNKS[c]
                xc = xch[c]
                ps = psum_pool.tile([P, D_OUT], f32, tag="ps", name=f"ps{j}")
                yt = y_pool.tile([P, D_OUT], bf16, tag="y", name=f"y{j}")
                tail = j >= TB - 2

                # schedule: hi pair 2 goes LAST so its weights (3rd in the
                # scalar ring's FIFO) have the most slack at startup.
                # (xc rows 0-5 = hi blocks, 6-9 = lo blocks.)
                SCHED = ((0, 0), (2, 1), (6, 0), (8, 1), (4, 2))

                # all matmuls of the 512-column half first, then the
                # 256-half: the 512-half PSUM group closes earlier so its
                # (bigger) copy-out overlaps the 256-half matmuls.
                def half(lo, hi_, wt):
                    for k, (row, m) in enumerate(SCHED):
                        nc.tensor.matmul(
                            ps[:, lo:hi_],
                            xc[:, row : row + 2, off : off + P],
                            wt[:, 2 * m : 2 * m + 2, :],
                            start=(k == 0),
                            stop=(k == len(SCHED) - 1),
                            perf_mode=DR,
                        )

                half(0, O_SPLIT, wat)
                if tail:
                    # tail: the 512-half copy+store launches while the PE
                    # still runs the 256-half matmuls; the (smaller)
                    # 256-half chain rides the ACT engine + its own ring,
                    # so only ~1us of copy/store latency trails the last
                    # matmul.
                    nc.vector.tensor_copy(yt[:, :O_SPLIT], ps[:, :O_SPLIT])
                    nc.sync.dma_start(
                        y[j * P : (j + 1) * P, :O_SPLIT], yt[:, :O_SPLIT]
                    )
                half(O_SPLIT, D_OUT, wbt)
                if tail:
                    # drain the final 256 columns as two 128-col pieces on
                    # parallel engines + rings, so the post-stream chain is
                    # one small copy + one descriptor-gen deep.
                    MID = O_SPLIT + 128
                    nc.scalar.copy(yt[:, O_SPLIT:MID], ps[:, O_SPLIT:MID])
                    nc.scalar.dma_start(
                        y[j * P : (j + 1) * P, O_SPLIT:MID], yt[:, O_SPLIT:MID]
                    )
                    nc.vector.tensor_copy(yt[:, MID:], ps[:, MID:])
                    nc.sync.dma_start(
                        y[j * P : (j + 1) * P, MID:], yt[:, MID:]
                    )
                else:
                    nc.vector.tensor_copy(yt[:, :O_SPLIT], ps[:, :O_SPLIT])
                    nc.scalar.copy(yt[:, O_SPLIT:], ps[:, O_SPLIT:])
                    eng = nc.sync if j % 2 == 0 else nc.scalar
                    eng.dma_start(y[j * P : (j + 1) * P, :], yt[:])

    nc.compile()
    return nc


def _get_nc():
    if "nc" not in _cache:
        _cache["nc"] = _build()
    return _cache["nc"]


def _pack_chunks(a):
    """[cores, blocks, 128, T] -> [cores, 128, blocks*T] with each token
    chunk's data contiguous per partition row (chunk-major, then
    block-major, token-minor) so chunk loads are 128 fat descriptors."""
    nco, nb, p, t = a.shape
    pieces = []
    s = 0
    for cw in CHUNKS:
        pieces.append(a[:, :, :, s : s + cw].transpose(0, 2, 1, 3)
                      .reshape(nco, p, nb * cw))
        s += cw
    return np.ascontiguousarray(np.concatenate(pieces, axis=2))


def _prep_inputs(x, weight):
    import ml_dtypes

    f8 = ml_dtypes.float8_e4m3
    x = np.asarray(x, dtype=np.float32)
    w = np.asarray(weight, dtype=np.float32)
    x2 = x.reshape(N_CORES, T_CORE, D_IN)
    # transpose so the contraction dim is on partitions, then hi/lo fp8 split
    xT = np.ascontiguousarray(x2.transpose(0, 2, 1))  # [8, 768, 4096] f32
    xT_hi = xT.astype(f8)
    xT_lo = (
        (xT[:, : LB * P, :] - xT_hi[:, : LB * P, :].astype(np.float32))
    ).astype(f8)
    comb = np.concatenate(
        [
            xT_hi.reshape(N_CORES, IB, P, T_CORE),
            xT_lo.reshape(N_CORES, LB, P, T_CORE),
        ],
        axis=1,
    )  # [cores, 10, 128, T]
    xp_packed = _pack_chunks(comb)
    # replicate the small binarized weight: +-1 (and 0) are exact in fp8.
    # pack partition-major, split into the 512- and 256-col halves.
    wT = np.sign(w).T.astype(f8)  # [i, o]
    wa = np.ascontiguousarray(
        wT[:, :O_SPLIT].reshape(IB, P, O_SPLIT).transpose(1, 0, 2)
        .reshape(P, IB * O_SPLIT)
    )
    wb = np.ascontiguousarray(
        wT[:, O_SPLIT:].reshape(IB, P, D_OUT - O_SPLIT).transpose(1, 0, 2)
        .reshape(P, IB * (D_OUT - O_SPLIT))
    )
    return [
        {"xp": xp_packed[c], "wa": wa, "wb": wb} for c in range(N_CORES)
    ]


def _install_axon_ntff_hook():
    """The agent image's `antenv` lacks `axon_hooks`; register an equivalent
    module backed by direct ctypes calls into libaxon_pjrt.so so that
    run_bass_kernel_spmd(trace=True) can capture NTFF profiles under axon."""
    import sys

    if "antenv.axon_hooks" in sys.modules:
        return
    import contextlib
    import ctypes
    import types

    so_path = "/opt/axon/libaxon_pjrt.so"
    try:
        lib = ctypes.CDLL(so_path)
    except OSError:
        return
    if not hasattr(lib, "axon_start_nrt_profile"):
        return
    lib.axon_start_nrt_profile.argtypes = [
        ctypes.POINTER(ctypes.c_int64),
        ctypes.c_size_t,
    ]
    lib.axon_start_nrt_profile.restype = ctypes.c_int64
    lib.axon_stop_nrt_profile.argtypes = [ctypes.c_char_p]
    lib.axon_stop_nrt_profile.restype = ctypes.c_int64

    @contextlib.contextmanager
    def _hook(output_dir, device_ids):
        import jax

        jax.devices()
        if device_ids:
            ids = (ctypes.c_int64 * len(device_ids))(*device_ids)
            rc = lib.axon_start_nrt_profile(ids, len(device_ids))
        else:
            rc = lib.axon_start_nrt_profile(None, 0)
        if rc != 0:
            raise RuntimeError(f"axon_start_nrt_profile rc={rc}")
        try:
            yield
        finally:
            n = lib.axon_stop_nrt_profile(str(output_dir).encode())
            print(f"ntff profile: {n} file(s) written to {output_dir}")

    mod = types.ModuleType("antenv.axon_hooks")
    mod.get_axon_ntff_profile_hook = lambda: _hook
    mod.set_axon_ntff_profile_hook = lambda h: None
    sys.modules["antenv.axon_hooks"] = mod


def _run(x, weight, trace=False):
    from concourse.bass_utils import run_bass_kernel_spmd

    if trace:
        _install_axon_ntff_hook()
    nc = _get_nc()
    in_maps = _prep_inputs(x, weight)
    res = run_bass_kernel_spmd(
        nc, in_maps, core_ids=list(range(N_CORES)), trace=trace
    )
    y_full = np.concatenate(
        [r["y"].astype(np.float32) for r in res.results], axis=0
    )
    return y_full.reshape(B, S, D_OUT), res


def kernel(x, weight):
    out, _ = _run(x, weight, trace=False)
    return out
